# revision 28
# baseline (speedup 1.0000x reference)
"""Trainium2 Bass kernel for dual-input multi-head attention.

Computes, for each of two independent inputs x, y of shape [8, 1024, 768]:
    qkv = inp @ w_qkv.T ; split into 12 heads of 64
    attn = softmax(q k^T / sqrt(64)) v
    out  = attn @ w_proj.T + b_proj
Sharded data-parallel over the batch dim: core i handles batch i of x AND
batch i of y (16 batch-units over 8 cores = 2 per core).

Per-core design:
  - Host pre-transposes and casts to bf16: inpT [C, N], w_qkvT [C, 3C],
    w_projT [C, C]. All matmuls run in bf16 (1 cycle/row on the PE) with
    fp32 PSUM accumulation.
  - QKV matmuls produce q,k TRANSPOSED ([head_dim, N] per head, as 12
    j-tiles of [128, N]) and v in natural [N, head_dim] layout with a
    column of ones appended, so the P@V matmul also emits the softmax
    denominator as a 65th output row for free.
  - Scores are computed transposed (pT[m, n] = k_m . q_n, contraction=64),
    exp on ScalarE straight out of PSUM (scale=1/sqrt(64) folded in; no
    max-subtraction — scores are O(+-15) so exp stays in fp32 range), and
    the P@V accumulation is interleaved with the score matmuls at key-tile
    granularity so the PE never idles long enough for the HAM clock gate
    to re-throttle it.
  - Normalization: the 4 denominator rows of a head-pair are gathered at
    partitions 0/32/64/96 of one tile and inverted by a single DVE
    reciprocal (its cost is per-free-element, so batching rows is ~4x
    cheaper); each inverse is staged back to a partition-0 row (GpSimd
    partition_broadcast only reads partition 0 on hardware), broadcast on
    the idle GpSimd engine, and applied with one multiply into the
    transposed attention-output buffer — exactly the lhsT layout the
    projection matmul wants.
  - Cross-input software pipelining: weights load once; QKV work of input
    y is drained from a filler queue between the attention head-passes of
    input x (filling the PE bubbles left by the ScalarE-bound softmax),
    and the projection of x likewise fills the attention of y.
"""

from collections import deque

import numpy as np

import concourse.bacc as bacc
import concourse.mybir as mybir
import concourse.tile as tile
from concourse import bass_utils

B, N, C, H, HD = 8, 1024, 768, 12, 64
NT = N // 128  # 8 token tiles
CT = C // 128  # 6 contraction chunks
SCALE = HD ** -0.5
F32 = mybir.dt.float32
BF16 = mybir.dt.bfloat16
AF = mybir.ActivationFunctionType
ALU = mybir.AluOpType
N_CORES = 8


def build_program():
    nc = bacc.Bacc("TRN2", target_bir_lowering=False, debug=False)
    inp_dram = [
        nc.dram_tensor("xT", [C, N], BF16, kind="ExternalInput"),
        nc.dram_tensor("yT", [C, N], BF16, kind="ExternalInput"),
    ]
    wqT = nc.dram_tensor("wqT", [C, 3 * C], BF16, kind="ExternalInput")
    wpT = nc.dram_tensor("wpT", [C, C], BF16, kind="ExternalInput")
    bp = nc.dram_tensor("bp", [1, C], F32, kind="ExternalInput")
    out_dram = [
        nc.dram_tensor("out_x", [N, C], F32, kind="ExternalOutput"),
        nc.dram_tensor("out_y", [N, C], F32, kind="ExternalOutput"),
    ]

    with tile.TileContext(nc) as tc:
        with (
            tc.tile_pool(name="pers", bufs=1) as pers,
            tc.tile_pool(name="dbl", bufs=2) as dbl,
            tc.tile_pool(name="pexp", bufs=4) as pep,
            tc.tile_pool(name="pvu", bufs=8) as pvup,
            tc.tile_pool(name="small", bufs=1) as smp,
            tc.tile_pool(name="rbsb", bufs=2) as rbsbp,
            tc.tile_pool(name="outp", bufs=2) as outp,
            tc.tile_pool(name="scps", bufs=2, space="PSUM") as scp,
            tc.tile_pool(name="mmps", bufs=4, space="PSUM") as mmp,
        ):
            # startup-critical DMAs first: interleave wq and x chunks so the
            # first QKV matmul group (needs wq[0] + x[0]) starts ASAP
            wq_sb = pers.tile([128, CT, 3 * C], BF16, name="wq_sb")
            inp_sb = {
                0: dbl.tile([128, CT, N], BF16, name="inp_sb", tag="inp"),
                1: dbl.tile([128, CT, N], BF16, name="inp_sb2", tag="inp"),
            }
            for c in range(CT):
                nc.sync.dma_start(wq_sb[:, c, :], wqT[c * 128 : (c + 1) * 128, :])
                nc.sync.dma_start(
                    inp_sb[0][:, c, :], inp_dram[0][c * 128 : (c + 1) * 128, :]
                )
            for c in range(CT):
                nc.sync.dma_start(
                    inp_sb[1][:, c, :], inp_dram[1][c * 128 : (c + 1) * 128, :]
                )
            wp_sb = pers.tile([128, CT, C], BF16, name="wp_sb")
            for c in range(CT):
                nc.sync.dma_start(wp_sb[:, c, :], wpT[c * 128 : (c + 1) * 128, :])
            b_row = pers.tile([1, C], F32, name="b_row")
            nc.sync.dma_start(b_row[:], bp[:, :])
            # bias folded into the projection matmul as a K=1 group-start:
            # psum = ones^T @ b + sum_c attnT^T @ w  (so the epilogue is a
            # plain copy that can run on whichever engine is idle)
            ones16 = pers.tile([1, 128], BF16, name="ones16")
            nc.vector.memset(ones16[:], 1.0)
            b16 = pers.tile([1, C], BF16, name="b16")
            nc.vector.tensor_copy(b16[:], b_row[:])

            qkT_sb, v_sb, attnT_sb = {}, {}, {}
            for idx in range(2):
                # q,k transposed: j-tiles 0..5 = q (2 heads/tile), 6..11 = k
                qkT_sb[idx] = dbl.tile([128, H, N], BF16, name="qkT_sb", tag="qkT")
                # v per (token-tile, head): 64 cols of v then one col of ones
                v_sb[idx] = dbl.tile([128, NT, H, HD + 1], BF16, name="v_sb", tag="v")
                nc.vector.memset(v_sb[idx][:, :, :, HD : HD + 1], 1.0)
                # attention output, transposed [C, N] as 6 chunks of 128
                attnT_sb[idx] = dbl.tile([128, CT, N], BF16, name="attnT_sb", tag="attnT")

            def emit_qkT(idx, jt, copy_engine, gs=(0, 1)):
                for _ in gen_qkT(idx, jt, copy_engine, gs):
                    pass

            def gen_qkT(idx, jt, copy_engine, gs=(0, 1)):
                # qkvT[j, n] = sum_c w_qkvT[c, j] inpT[c, n]
                for g in gs:
                    ps = mmp.tile([128, 512], F32, name="ps_qk", tag="mm")
                    for c in range(CT):
                        nc.tensor.matmul(
                            ps[:],
                            wq_sb[:, c, jt * 128 : (jt + 1) * 128],
                            inp_sb[idx][:, c, g * 512 : (g + 1) * 512],
                            start=(c == 0),
                            stop=(c == CT - 1),
                        )
                        yield
                    dst = qkT_sb[idx][:, jt, g * 512 : (g + 1) * 512]
                    if copy_engine == "act":
                        nc.scalar.copy(dst, ps[:])
                    else:
                        nc.vector.tensor_copy(dst, ps[:])

            def emit_v(idx, nt, g, copy_engine):
                for _ in gen_v(idx, nt, g, copy_engine):
                    pass

            def gen_v(idx, nt, g, copy_engine):
                # v[n, j] = sum_c inpT[c, n] w_qkvT[c, 2C + j]
                w = 512 if g == 0 else 256
                ps = mmp.tile([128, 512], F32, name="ps_v", tag="mm")
                for c in range(CT):
                    nc.tensor.matmul(
                        ps[:, :w],
                        inp_sb[idx][:, c, nt * 128 : (nt + 1) * 128],
                        wq_sb[:, c, 2 * C + g * 512 : 2 * C + g * 512 + w],
                        start=(c == 0),
                        stop=(c == CT - 1),
                    )
                    yield
                hview = ps[:, :w].rearrange("p (h d) -> p h d", d=HD)
                dst = v_sb[idx][:, nt, g * 8 : g * 8 + w // HD, 0:HD]
                if copy_engine == "act":
                    nc.scalar.copy(dst, hview)
                else:
                    nc.vector.tensor_copy(dst, hview)

            def emit_proj(idx, nt, copy_engine="dve"):
                for _ in gen_proj(idx, nt, copy_engine):
                    pass

            def gen_proj(idx, nt, copy_engine="dve"):
                p1 = mmp.tile([128, 512], F32, name="p1", tag="mm")
                p2 = mmp.tile([128, 512], F32, name="p2", tag="mm")
                nc.tensor.matmul(p1[:], ones16[:1, :], b16[:1, 0:512], start=True, stop=False)
                for c in range(CT):
                    nc.tensor.matmul(
                        p1[:],
                        attnT_sb[idx][:, c, nt * 128 : (nt + 1) * 128],
                        wp_sb[:, c, 0:512],
                        start=False,
                        stop=(c == CT - 1),
                    )
                    yield
                nc.tensor.matmul(p2[:, :256], ones16[:1, :], b16[:1, 512:768], start=True, stop=False)
                for c in range(CT):
                    nc.tensor.matmul(
                        p2[:, :256],
                        attnT_sb[idx][:, c, nt * 128 : (nt + 1) * 128],
                        wp_sb[:, c, 512:768],
                        start=False,
                        stop=(c == CT - 1),
                    )
                    yield
                out_sb = outp.tile([128, C], F32, name="out_sb", tag="outsb")
                if copy_engine == "act":
                    nc.scalar.copy(out_sb[:, 0:512], p1[:])
                    nc.scalar.copy(out_sb[:, 512:768], p2[:, :256])
                else:
                    nc.vector.tensor_copy(out_sb[:, 0:512], p1[:])
                    nc.vector.tensor_copy(out_sb[:, 512:768], p2[:, :256])
                nc.sync.dma_start(out_dram[idx][nt * 128 : (nt + 1) * 128, :], out_sb[:])

            fillers = deque()  # generators yielding once per PE matmul

            def drain_mm(k):
                # advance filler work by k PE matmuls
                while k > 0 and fillers:
                    try:
                        next(fillers[0])
                        k -= 1
                    except StopIteration:
                        fillers.popleft()

            def drain_all():
                while fillers:
                    try:
                        next(fillers[0])
                    except StopIteration:
                        fillers.popleft()

            def emit_norm(idx, t, pvu):
                # batched softmax denominators: gather at partitions 0/32/64/96,
                # one reciprocal, stage each row back to partition 0 (GpSimd
                # partition_broadcast only reads partition 0 on HW)
                keys = list(pvu)
                sums4 = smp.tile([128, 512], F32, name="sums4", tag="sums")
                nc.vector.memset(sums4[:], 1.0)
                for r, k in enumerate(keys):
                    nc.vector.tensor_copy(
                        sums4[32 * r : 32 * r + 1, :], pvu[k][HD : HD + 1, :]
                    )
                recip4 = smp.tile([128, 512], F32, name="recip4", tag="recip")
                nc.vector.reciprocal(recip4[0:97, :], sums4[0:97, :])
                for r, (ab, g) in enumerate(keys):
                    h = 2 * t + ab
                    hc, pb = h // 2, (h % 2) * 64
                    if r == 0:
                        stage = recip4
                    else:
                        stage = smp.tile([1, 512], F32, name=f"st{r}", tag=f"st{r}")
                        nc.vector.tensor_copy(stage[0:1, :], recip4[32 * r : 32 * r + 1, :])
                    rb_sb = rbsbp.tile([64, 512], F32, name="rb_sb", tag="rb")
                    nc.gpsimd.partition_broadcast(rb_sb[:], stage[0:1, :])
                    nc.vector.tensor_tensor(
                        attnT_sb[idx][pb : pb + 64, hc, g * 512 : (g + 1) * 512],
                        pvu[(ab, g)][0:HD, :],
                        rb_sb[:],
                        op=ALU.mult,
                    )

            def emit_attn_pair(idx, t, prev_norm=None, last=False):
                # Two passes over the key tiles, one per 512-col n-half.
                # Within a pass both heads advance together: their
                # contraction-64 score matmuls sit in disjoint PE row groups
                # (partitions 0-63 / 64-127) and adjacent emission makes the
                # hardware run each pair concurrently (measured 2x). A pass
                # holds only 2 P@V accumulators, leaving 2 PSUM slots in the
                # shared pool for the cross-input filler work.
                pvu = {}
                for g in range(2):
                    if g == 1 and prev_norm is not None:
                        # previous pair's normalization sits mid-pair on the
                        # DVE so pair boundaries only carry the pvu copies
                        emit_norm(*prev_norm)
                    pv = {
                        ab: mmp.tile([HD + 1, 512], F32, name="pv", tag="mm")
                        for ab in range(2)
                    }
                    for mt in range(NT):
                        # one tile holds this n-half's scores for BOTH heads
                        sc = scp.tile([128, 2, 512], F32, name="sc", tag="sc")
                        for ab in range(2):
                            pb = ab * 64
                            nc.tensor.matmul(
                                sc[:, ab, :],
                                qkT_sb[idx][pb : pb + 64, 6 + t, mt * 128 : (mt + 1) * 128],
                                qkT_sb[idx][pb : pb + 64, t, g * 512 : (g + 1) * 512],
                                start=True,
                                stop=True,
                                tile_position=(pb, 0),
                            )
                        pe = pep.tile([128, N], BF16, name="pe", tag="pexp")
                        nc.scalar.activation(
                            pe[:],
                            sc[:].rearrange("p a b -> p (a b)"),
                            AF.Exp,
                            scale=SCALE,
                        )
                        # interleave P@V partial sums with the score stream
                        for ab in range(2):
                            nc.tensor.matmul(
                                pv[ab],
                                v_sb[idx][:, mt, 2 * t + ab, :],
                                pe[:, ab * 512 : (ab + 1) * 512],
                                start=(mt == 0),
                                stop=(mt == NT - 1),
                            )
                        drain_mm(2)  # smooth background PE work per key tile
                    for ab in range(2):
                        u = pvup.tile([HD + 1, 512], F32, name="pvu", tag="pvu")
                        # last pair: copy on ScalarE (idle by then) so the DVE
                        # queue doesn't delay releasing the PSUM accumulators
                        if last:
                            nc.scalar.copy(u[:], pv[ab][:])
                        else:
                            nc.vector.tensor_copy(u[:], pv[ab][:])
                        pvu[(ab, g)] = u
                    drain_mm(4)
                return (idx, t, pvu)

            # ---- pipelined emission ----
            # prologue: only what attn(x) pair 0 needs -- q/k j-tiles 0 and
            # 6 plus the 512-col v chunks (heads 0-7); the rest of QKV(x)
            # drains as filler inside the attention windows
            emit_qkT(0, 0, "act")
            emit_qkT(0, 6, "act")
            for nt in range(NT):
                emit_v(0, nt, 0, "act")
            # pair t+1's j-tiles head the queue so pair t's drains emit them
            for t in range(1, H // 2):
                fillers.append(gen_qkT(0, t, "dve"))
                fillers.append(gen_qkT(0, 6 + t, "dve"))
            for nt in range(NT):
                fillers.append(gen_v(0, nt, 1, "dve"))
            for jt in range(H):
                fillers.append(gen_qkT(1, jt, "dve"))
            for nt in range(NT):
                fillers.append(gen_v(1, nt, 0, "dve"))
            pend_norm = None
            for t in range(H // 2):
                pend_norm = emit_attn_pair(0, t, prev_norm=pend_norm, last=(t == H // 2 - 1))
            drain_all()
            # attn(y) with v(y) tail chunks + proj(x) drained in
            for nt in range(NT):
                fillers.append(gen_v(1, nt, 1, "dve"))
            for nt in range(NT):
                fillers.append(gen_proj(0, nt))
            for t in range(H // 2):
                pend_norm = emit_attn_pair(1, t, prev_norm=pend_norm, last=(t == H // 2 - 1))
            emit_norm(*pend_norm)
            drain_all()
            for nt in range(NT):
                emit_proj(1, nt, copy_engine="act")

    nc.compile()
    return nc


_PROGRAM = None


def _get_program():
    global _PROGRAM
    if _PROGRAM is None:
        _PROGRAM = build_program()
    return _PROGRAM


def make_in_maps(x, y, w_qkv, w_proj, b_proj):
    import ml_dtypes

    bf = ml_dtypes.bfloat16
    x = np.asarray(x, np.float32)
    y = np.asarray(y, np.float32)
    xT = np.ascontiguousarray(x.transpose(0, 2, 1)).astype(bf)
    yT = np.ascontiguousarray(y.transpose(0, 2, 1)).astype(bf)
    wqT = np.ascontiguousarray(np.asarray(w_qkv, np.float32).T).astype(bf)
    wpT = np.ascontiguousarray(np.asarray(w_proj, np.float32).T).astype(bf)
    bp = np.ascontiguousarray(np.asarray(b_proj, np.float32).reshape(1, C))
    return [
        {"xT": xT[i], "yT": yT[i], "wqT": wqT, "wpT": wpT, "bp": bp}
        for i in range(N_CORES)
    ]


def kernel(x, y, w_qkv, w_proj, b_proj):
    nc = _get_program()
    in_maps = make_in_maps(x, y, w_qkv, w_proj, b_proj)
    res = bass_utils.run_bass_kernel_spmd(nc, in_maps, core_ids=list(range(N_CORES)))
    xo = np.stack([np.asarray(res.results[i]["out_x"]) for i in range(N_CORES)])
    yo = np.stack([np.asarray(res.results[i]["out_y"]) for i in range(N_CORES)])
    return (xo, yo)


# revision 29
# speedup vs baseline: 1.1977x; 1.1977x over previous
"""Trainium2 Bass kernel for dual-input multi-head attention.

Computes, for each of two independent inputs x, y of shape [8, 1024, 768]:
    qkv = inp @ w_qkv.T ; split into 12 heads of 64
    attn = softmax(q k^T / sqrt(64)) v
    out  = attn @ w_proj.T + b_proj
Sharded data-parallel over the batch dim: core i handles batch i of x AND
batch i of y (16 batch-units over 8 cores = 2 per core).

Per-core design:
  - Host pre-transposes and casts to bf16: inpT [C, N], w_qkvT [C, 3C],
    w_projT [C, C]. All matmuls run in bf16 (1 cycle/row on the PE) with
    fp32 PSUM accumulation.
  - QKV matmuls produce q,k TRANSPOSED ([head_dim, N] per head, as 12
    j-tiles of [128, N]) and v in natural [N, head_dim] layout with a
    column of ones appended, so the P@V matmul also emits the softmax
    denominator as a 65th output row for free.
  - Scores are computed transposed (pT[m, n] = k_m . q_n, contraction=64),
    exp on ScalarE straight out of PSUM (scale=1/sqrt(64) folded in; no
    max-subtraction — scores are O(+-15) so exp stays in fp32 range), and
    the P@V accumulation is interleaved with the score matmuls at key-tile
    granularity so the PE never idles long enough for the HAM clock gate
    to re-throttle it.
  - Normalization: the 4 denominator rows of a head-pair are gathered at
    partitions 0/32/64/96 of one tile and inverted by a single DVE
    reciprocal (its cost is per-free-element, so batching rows is ~4x
    cheaper); each inverse is staged back to a partition-0 row (GpSimd
    partition_broadcast only reads partition 0 on hardware), broadcast on
    the idle GpSimd engine, and applied with one multiply into the
    transposed attention-output buffer — exactly the lhsT layout the
    projection matmul wants.
  - Cross-input software pipelining: weights load once; QKV work of input
    y is drained from a filler queue between the attention head-passes of
    input x (filling the PE bubbles left by the ScalarE-bound softmax),
    and the projection of x likewise fills the attention of y.
"""

from collections import deque

import numpy as np

import concourse.bacc as bacc
import concourse.mybir as mybir
import concourse.tile as tile
from concourse import bass_utils

B, N, C, H, HD = 8, 1024, 768, 12, 64
NT = N // 128  # 8 token tiles
CT = C // 128  # 6 contraction chunks
SCALE = HD ** -0.5
F32 = mybir.dt.float32
BF16 = mybir.dt.bfloat16
AF = mybir.ActivationFunctionType
ALU = mybir.AluOpType
N_CORES = 8


def build_program():
    nc = bacc.Bacc("TRN2", target_bir_lowering=False, debug=False)
    inp_dram = [
        nc.dram_tensor("xT", [C, N], BF16, kind="ExternalInput"),
        nc.dram_tensor("yT", [C, N], BF16, kind="ExternalInput"),
    ]
    wqT = nc.dram_tensor("wqT", [C, 3 * C], BF16, kind="ExternalInput")
    wpT = nc.dram_tensor("wpT", [C, C], BF16, kind="ExternalInput")
    bp = nc.dram_tensor("bp", [1, C], F32, kind="ExternalInput")
    out_dram = [
        nc.dram_tensor("out_x", [N, C], F32, kind="ExternalOutput"),
        nc.dram_tensor("out_y", [N, C], F32, kind="ExternalOutput"),
    ]

    with tile.TileContext(nc) as tc:
        with (
            tc.tile_pool(name="pers", bufs=1) as pers,
            tc.tile_pool(name="dbl", bufs=2) as dbl,
            tc.tile_pool(name="pexp", bufs=4) as pep,
            tc.tile_pool(name="pvu", bufs=8) as pvup,
            tc.tile_pool(name="small", bufs=1) as smp,
            tc.tile_pool(name="rbsb", bufs=2) as rbsbp,
            tc.tile_pool(name="outp", bufs=2) as outp,
            tc.tile_pool(name="scps", bufs=2, space="PSUM") as scp,
            tc.tile_pool(name="mmps", bufs=4, space="PSUM") as mmp,
        ):
            # startup-critical DMAs first: interleave wq and x chunks so the
            # first QKV matmul group (needs wq[0] + x[0]) starts ASAP
            wq_sb = pers.tile([128, CT, 3 * C], BF16, name="wq_sb")
            inp_sb = {
                0: dbl.tile([128, CT, N], BF16, name="inp_sb", tag="inp"),
                1: dbl.tile([128, CT, N], BF16, name="inp_sb2", tag="inp"),
            }
            for c in range(CT):
                nc.sync.dma_start(wq_sb[:, c, :], wqT[c * 128 : (c + 1) * 128, :])
                nc.sync.dma_start(
                    inp_sb[0][:, c, :], inp_dram[0][c * 128 : (c + 1) * 128, :]
                )
            for c in range(CT):
                nc.sync.dma_start(
                    inp_sb[1][:, c, :], inp_dram[1][c * 128 : (c + 1) * 128, :]
                )
            wp_sb = pers.tile([128, CT, C], BF16, name="wp_sb")
            for c in range(CT):
                nc.sync.dma_start(wp_sb[:, c, :], wpT[c * 128 : (c + 1) * 128, :])
            b_row = pers.tile([1, C], F32, name="b_row")
            nc.sync.dma_start(b_row[:], bp[:, :])
            bias_sb = pers.tile([128, C], F32, name="bias_sb")
            nc.gpsimd.partition_broadcast(bias_sb[:], b_row[:1, :])

            qkT_sb, v_sb, attnT_sb = {}, {}, {}
            for idx in range(2):
                # q,k transposed: j-tiles 0..5 = q (2 heads/tile), 6..11 = k
                qkT_sb[idx] = dbl.tile([128, H, N], BF16, name="qkT_sb", tag="qkT")
                # v per (token-tile, head): 64 cols of v then one col of ones
                v_sb[idx] = dbl.tile([128, NT, H, HD + 1], BF16, name="v_sb", tag="v")
                nc.vector.memset(v_sb[idx][:, :, :, HD : HD + 1], 1.0)
                # attention output, transposed [C, N] as 6 chunks of 128
                attnT_sb[idx] = dbl.tile([128, CT, N], BF16, name="attnT_sb", tag="attnT")

            def emit_qkT(idx, jt, copy_engine, gs=(0, 1)):
                for _ in gen_qkT(idx, jt, copy_engine, gs):
                    pass

            def gen_qkT(idx, jt, copy_engine, gs=(0, 1)):
                # qkvT[j, n] = sum_c w_qkvT[c, j] inpT[c, n]
                for g in gs:
                    ps = mmp.tile([128, 512], F32, name="ps_qk", tag="mm")
                    for c in range(CT):
                        nc.tensor.matmul(
                            ps[:],
                            wq_sb[:, c, jt * 128 : (jt + 1) * 128],
                            inp_sb[idx][:, c, g * 512 : (g + 1) * 512],
                            start=(c == 0),
                            stop=(c == CT - 1),
                        )
                        yield
                    dst = qkT_sb[idx][:, jt, g * 512 : (g + 1) * 512]
                    if copy_engine == "act":
                        nc.scalar.copy(dst, ps[:])
                    else:
                        nc.vector.tensor_copy(dst, ps[:])

            def emit_v(idx, nt, g, copy_engine):
                for _ in gen_v(idx, nt, g, copy_engine):
                    pass

            def gen_v(idx, nt, g, copy_engine):
                # v[n, j] = sum_c inpT[c, n] w_qkvT[c, 2C + j]
                w = 512 if g == 0 else 256
                ps = mmp.tile([128, 512], F32, name="ps_v", tag="mm")
                for c in range(CT):
                    nc.tensor.matmul(
                        ps[:, :w],
                        inp_sb[idx][:, c, nt * 128 : (nt + 1) * 128],
                        wq_sb[:, c, 2 * C + g * 512 : 2 * C + g * 512 + w],
                        start=(c == 0),
                        stop=(c == CT - 1),
                    )
                    yield
                hview = ps[:, :w].rearrange("p (h d) -> p h d", d=HD)
                dst = v_sb[idx][:, nt, g * 8 : g * 8 + w // HD, 0:HD]
                if copy_engine == "act":
                    nc.scalar.copy(dst, hview)
                else:
                    nc.vector.tensor_copy(dst, hview)

            def emit_proj(idx, nt):
                for _ in gen_proj(idx, nt):
                    pass

            def gen_proj(idx, nt):
                p1 = mmp.tile([128, 512], F32, name="p1", tag="mm")
                p2 = mmp.tile([128, 512], F32, name="p2", tag="mm")
                for c in range(CT):
                    nc.tensor.matmul(
                        p1[:],
                        attnT_sb[idx][:, c, nt * 128 : (nt + 1) * 128],
                        wp_sb[:, c, 0:512],
                        start=(c == 0),
                        stop=(c == CT - 1),
                    )
                    yield
                for c in range(CT):
                    nc.tensor.matmul(
                        p2[:, :256],
                        attnT_sb[idx][:, c, nt * 128 : (nt + 1) * 128],
                        wp_sb[:, c, 512:768],
                        start=(c == 0),
                        stop=(c == CT - 1),
                    )
                    yield
                out_sb = outp.tile([128, C], F32, name="out_sb", tag="outsb")
                nc.vector.tensor_tensor(
                    out_sb[:, 0:512], p1[:], bias_sb[:, 0:512], op=ALU.add
                )
                nc.vector.tensor_tensor(
                    out_sb[:, 512:768], p2[:, :256], bias_sb[:, 512:768], op=ALU.add
                )
                nc.sync.dma_start(out_dram[idx][nt * 128 : (nt + 1) * 128, :], out_sb[:])

            fillers = deque()  # generators yielding once per PE matmul

            def drain_mm(k):
                # advance filler work by k PE matmuls
                while k > 0 and fillers:
                    try:
                        next(fillers[0])
                        k -= 1
                    except StopIteration:
                        fillers.popleft()

            def drain_all():
                while fillers:
                    try:
                        next(fillers[0])
                    except StopIteration:
                        fillers.popleft()

            def emit_norm(idx, t, pvu):
                # batched softmax denominators: gather at partitions 0/32/64/96,
                # one reciprocal, stage each row back to partition 0 (GpSimd
                # partition_broadcast only reads partition 0 on HW)
                keys = list(pvu)
                sums4 = smp.tile([128, 512], F32, name="sums4", tag="sums")
                nc.vector.memset(sums4[:], 1.0)
                for r, k in enumerate(keys):
                    nc.vector.tensor_copy(
                        sums4[32 * r : 32 * r + 1, :], pvu[k][HD : HD + 1, :]
                    )
                recip4 = smp.tile([128, 512], F32, name="recip4", tag="recip")
                nc.vector.reciprocal(recip4[0:97, :], sums4[0:97, :])
                for r, (ab, g) in enumerate(keys):
                    h = 2 * t + ab
                    hc, pb = h // 2, (h % 2) * 64
                    if r == 0:
                        stage = recip4
                    else:
                        stage = smp.tile([1, 512], F32, name=f"st{r}", tag=f"st{r}")
                        nc.vector.tensor_copy(stage[0:1, :], recip4[32 * r : 32 * r + 1, :])
                    rb_sb = rbsbp.tile([64, 512], F32, name="rb_sb", tag="rb")
                    nc.gpsimd.partition_broadcast(rb_sb[:], stage[0:1, :])
                    nc.vector.tensor_tensor(
                        attnT_sb[idx][pb : pb + 64, hc, g * 512 : (g + 1) * 512],
                        pvu[(ab, g)][0:HD, :],
                        rb_sb[:],
                        op=ALU.mult,
                    )

            def emit_attn_pair(idx, t, prev_norm=None, last=False):
                # Two passes over the key tiles, one per 512-col n-half.
                # Within a pass both heads advance together: their
                # contraction-64 score matmuls sit in disjoint PE row groups
                # (partitions 0-63 / 64-127) and adjacent emission makes the
                # hardware run each pair concurrently (measured 2x). A pass
                # holds only 2 P@V accumulators, leaving 2 PSUM slots in the
                # shared pool for the cross-input filler work.
                pvu = {}
                for g in range(2):
                    if g == 1 and prev_norm is not None:
                        # previous pair's normalization sits mid-pair on the
                        # DVE so pair boundaries only carry the pvu copies
                        emit_norm(*prev_norm)
                    pv = {
                        ab: mmp.tile([HD + 1, 512], F32, name="pv", tag="mm")
                        for ab in range(2)
                    }
                    for mt in range(NT):
                        # one tile holds this n-half's scores for BOTH heads
                        sc = scp.tile([128, 2, 512], F32, name="sc", tag="sc")
                        for ab in range(2):
                            pb = ab * 64
                            nc.tensor.matmul(
                                sc[:, ab, :],
                                qkT_sb[idx][pb : pb + 64, 6 + t, mt * 128 : (mt + 1) * 128],
                                qkT_sb[idx][pb : pb + 64, t, g * 512 : (g + 1) * 512],
                                start=True,
                                stop=True,
                                tile_position=(pb, 0),
                            )
                        pe = pep.tile([128, N], BF16, name="pe", tag="pexp")
                        nc.scalar.activation(
                            pe[:],
                            sc[:].rearrange("p a b -> p (a b)"),
                            AF.Exp,
                            scale=SCALE,
                        )
                        # interleave P@V partial sums with the score stream
                        for ab in range(2):
                            nc.tensor.matmul(
                                pv[ab],
                                v_sb[idx][:, mt, 2 * t + ab, :],
                                pe[:, ab * 512 : (ab + 1) * 512],
                                start=(mt == 0),
                                stop=(mt == NT - 1),
                            )
                        drain_mm(2)  # smooth background PE work per key tile
                    for ab in range(2):
                        u = pvup.tile([HD + 1, 512], F32, name="pvu", tag="pvu")
                        # last pair: copy on ScalarE (idle by then) so the DVE
                        # queue doesn't delay releasing the PSUM accumulators
                        if last:
                            nc.scalar.copy(u[:], pv[ab][:])
                        else:
                            nc.vector.tensor_copy(u[:], pv[ab][:])
                        pvu[(ab, g)] = u
                    drain_mm(4)
                return (idx, t, pvu)

            # ---- pipelined emission ----
            # prologue: only what attn(x) pair 0 needs -- q/k j-tiles 0 and
            # 6 plus the 512-col v chunks (heads 0-7); the rest of QKV(x)
            # drains as filler inside the attention windows
            emit_qkT(0, 0, "act")
            emit_qkT(0, 6, "act")
            for nt in range(NT):
                emit_v(0, nt, 0, "act")
            # pair t+1's j-tiles head the queue so pair t's drains emit them
            for t in range(1, H // 2):
                fillers.append(gen_qkT(0, t, "dve"))
                fillers.append(gen_qkT(0, 6 + t, "dve"))
            for nt in range(NT):
                fillers.append(gen_v(0, nt, 1, "dve"))
            for jt in range(H):
                fillers.append(gen_qkT(1, jt, "dve"))
            for nt in range(NT):
                fillers.append(gen_v(1, nt, 0, "dve"))
            pend_norm = None
            for t in range(H // 2):
                pend_norm = emit_attn_pair(0, t, prev_norm=pend_norm, last=(t == H // 2 - 1))
            drain_all()
            # attn(y) with v(y) tail chunks + proj(x) drained in
            for nt in range(NT):
                fillers.append(gen_v(1, nt, 1, "dve"))
            for nt in range(NT):
                fillers.append(gen_proj(0, nt))
            for t in range(H // 2):
                pend_norm = emit_attn_pair(1, t, prev_norm=pend_norm, last=(t == H // 2 - 1))
            emit_norm(*pend_norm)
            drain_all()
            for nt in range(NT):
                emit_proj(1, nt)

    nc.compile()
    return nc


_PROGRAM = None


def _get_program():
    global _PROGRAM
    if _PROGRAM is None:
        _PROGRAM = build_program()
    return _PROGRAM


def make_in_maps(x, y, w_qkv, w_proj, b_proj):
    import ml_dtypes

    bf = ml_dtypes.bfloat16
    x = np.asarray(x, np.float32)
    y = np.asarray(y, np.float32)
    xT = np.ascontiguousarray(x.transpose(0, 2, 1)).astype(bf)
    yT = np.ascontiguousarray(y.transpose(0, 2, 1)).astype(bf)
    wqT = np.ascontiguousarray(np.asarray(w_qkv, np.float32).T).astype(bf)
    wpT = np.ascontiguousarray(np.asarray(w_proj, np.float32).T).astype(bf)
    bp = np.ascontiguousarray(np.asarray(b_proj, np.float32).reshape(1, C))
    return [
        {"xT": xT[i], "yT": yT[i], "wqT": wqT, "wpT": wpT, "bp": bp}
        for i in range(N_CORES)
    ]


def kernel(x, y, w_qkv, w_proj, b_proj):
    nc = _get_program()
    in_maps = make_in_maps(x, y, w_qkv, w_proj, b_proj)
    res = bass_utils.run_bass_kernel_spmd(nc, in_maps, core_ids=list(range(N_CORES)))
    xo = np.stack([np.asarray(res.results[i]["out_x"]) for i in range(N_CORES)])
    yo = np.stack([np.asarray(res.results[i]["out_y"]) for i in range(N_CORES)])
    return (xo, yo)


# revision 30
# speedup vs baseline: 1.2044x; 1.0056x over previous
"""Trainium2 Bass kernel for dual-input multi-head attention.

Computes, for each of two independent inputs x, y of shape [8, 1024, 768]:
    qkv = inp @ w_qkv.T ; split into 12 heads of 64
    attn = softmax(q k^T / sqrt(64)) v
    out  = attn @ w_proj.T + b_proj
Sharded data-parallel over the batch dim: core i handles batch i of x AND
batch i of y (16 batch-units over 8 cores = 2 per core).

Per-core design:
  - Host pre-transposes and casts to bf16: inpT [C, N], w_qkvT [C, 3C],
    w_projT [C, C]. All matmuls run in bf16 (1 cycle/row on the PE) with
    fp32 PSUM accumulation.
  - QKV matmuls produce q,k TRANSPOSED ([head_dim, N] per head, as 12
    j-tiles of [128, N]) and v in natural [N, head_dim] layout with a
    column of ones appended, so the P@V matmul also emits the softmax
    denominator as a 65th output row for free.
  - Scores are computed transposed (pT[m, n] = k_m . q_n, contraction=64),
    exp on ScalarE straight out of PSUM (scale=1/sqrt(64) folded in; no
    max-subtraction — scores are O(+-15) so exp stays in fp32 range), and
    the P@V accumulation is interleaved with the score matmuls at key-tile
    granularity so the PE never idles long enough for the HAM clock gate
    to re-throttle it.
  - Normalization: the 4 denominator rows of a head-pair are gathered at
    partitions 0/32/64/96 of one tile and inverted by a single DVE
    reciprocal (its cost is per-free-element, so batching rows is ~4x
    cheaper); each inverse is staged back to a partition-0 row (GpSimd
    partition_broadcast only reads partition 0 on hardware), broadcast on
    the idle GpSimd engine, and applied with one multiply into the
    transposed attention-output buffer — exactly the lhsT layout the
    projection matmul wants.
  - Cross-input software pipelining: weights load once; QKV work of input
    y is drained from a filler queue between the attention head-passes of
    input x (filling the PE bubbles left by the ScalarE-bound softmax),
    and the projection of x likewise fills the attention of y.
"""

from collections import deque

import numpy as np

import concourse.bacc as bacc
import concourse.mybir as mybir
import concourse.tile as tile
from concourse import bass_utils

B, N, C, H, HD = 8, 1024, 768, 12, 64
NT = N // 128  # 8 token tiles
CT = C // 128  # 6 contraction chunks
SCALE = HD ** -0.5
F32 = mybir.dt.float32
BF16 = mybir.dt.bfloat16
AF = mybir.ActivationFunctionType
ALU = mybir.AluOpType
N_CORES = 8


def build_program():
    nc = bacc.Bacc("TRN2", target_bir_lowering=False, debug=False)
    inp_dram = [
        nc.dram_tensor("xT", [C, N], BF16, kind="ExternalInput"),
        nc.dram_tensor("yT", [C, N], BF16, kind="ExternalInput"),
    ]
    wqT = nc.dram_tensor("wqT", [C, 3 * C], BF16, kind="ExternalInput")
    wpT = nc.dram_tensor("wpT", [C, C], BF16, kind="ExternalInput")
    bp = nc.dram_tensor("bp", [1, C], F32, kind="ExternalInput")
    out_dram = [
        nc.dram_tensor("out_x", [N, C], F32, kind="ExternalOutput"),
        nc.dram_tensor("out_y", [N, C], F32, kind="ExternalOutput"),
    ]

    with tile.TileContext(nc) as tc:
        with (
            tc.tile_pool(name="pers", bufs=1) as pers,
            tc.tile_pool(name="dbl", bufs=2) as dbl,
            tc.tile_pool(name="pexp", bufs=4) as pep,
            tc.tile_pool(name="pvu", bufs=8) as pvup,
            tc.tile_pool(name="small", bufs=1) as smp,
            tc.tile_pool(name="rbsb", bufs=2) as rbsbp,
            tc.tile_pool(name="outp", bufs=2) as outp,
            tc.tile_pool(name="scps", bufs=2, space="PSUM") as scp,
            tc.tile_pool(name="mmps", bufs=4, space="PSUM") as mmp,
        ):
            # startup-critical DMAs first: interleave wq and x chunks so the
            # first QKV matmul group (needs wq[0] + x[0]) starts ASAP
            wq_sb = pers.tile([128, CT, 3 * C], BF16, name="wq_sb")
            inp_sb = {
                0: dbl.tile([128, CT, N], BF16, name="inp_sb", tag="inp"),
                1: dbl.tile([128, CT, N], BF16, name="inp_sb2", tag="inp"),
            }
            for c in range(CT):
                nc.sync.dma_start(wq_sb[:, c, :], wqT[c * 128 : (c + 1) * 128, :])
                nc.sync.dma_start(
                    inp_sb[0][:, c, :], inp_dram[0][c * 128 : (c + 1) * 128, :]
                )
            for c in range(CT):
                nc.sync.dma_start(
                    inp_sb[1][:, c, :], inp_dram[1][c * 128 : (c + 1) * 128, :]
                )
            wp_sb = pers.tile([128, CT, C], BF16, name="wp_sb")
            for c in range(CT):
                nc.sync.dma_start(wp_sb[:, c, :], wpT[c * 128 : (c + 1) * 128, :])
            b_row = pers.tile([1, C], F32, name="b_row")
            nc.sync.dma_start(b_row[:], bp[:, :])
            bias_sb = pers.tile([128, C], F32, name="bias_sb")
            nc.gpsimd.partition_broadcast(bias_sb[:], b_row[:1, :])

            qkT_sb, v_sb, attnT_sb = {}, {}, {}
            for idx in range(2):
                # q,k transposed: j-tiles 0..5 = q (2 heads/tile), 6..11 = k
                qkT_sb[idx] = dbl.tile([128, H, N], BF16, name="qkT_sb", tag="qkT")
                # v per (token-tile, head): 64 cols of v then one col of ones
                v_sb[idx] = dbl.tile([128, NT, H, HD + 1], BF16, name="v_sb", tag="v")
                nc.vector.memset(v_sb[idx][:, :, :, HD : HD + 1], 1.0)
                # attention output, transposed [C, N] as 6 chunks of 128
                attnT_sb[idx] = dbl.tile([128, CT, N], BF16, name="attnT_sb", tag="attnT")

            def emit_qkT(idx, jt, copy_engine, gs=(0, 1)):
                for _ in gen_qkT(idx, jt, copy_engine, gs):
                    pass

            def gen_qkT(idx, jt, copy_engine, gs=(0, 1)):
                # qkvT[j, n] = sum_c w_qkvT[c, j] inpT[c, n]
                for g in gs:
                    ps = mmp.tile([128, 512], F32, name="ps_qk", tag="mm")
                    for c in range(CT):
                        nc.tensor.matmul(
                            ps[:],
                            wq_sb[:, c, jt * 128 : (jt + 1) * 128],
                            inp_sb[idx][:, c, g * 512 : (g + 1) * 512],
                            start=(c == 0),
                            stop=(c == CT - 1),
                        )
                        yield
                    dst = qkT_sb[idx][:, jt, g * 512 : (g + 1) * 512]
                    if copy_engine == "act":
                        nc.scalar.copy(dst, ps[:])
                    else:
                        nc.vector.tensor_copy(dst, ps[:])

            def emit_v(idx, nt, g, copy_engine):
                for _ in gen_v(idx, nt, g, copy_engine):
                    pass

            def gen_v(idx, nt, g, copy_engine):
                # v[n, j] = sum_c inpT[c, n] w_qkvT[c, 2C + j]
                w = 512 if g == 0 else 256
                ps = mmp.tile([128, 512], F32, name="ps_v", tag="mm")
                for c in range(CT):
                    nc.tensor.matmul(
                        ps[:, :w],
                        inp_sb[idx][:, c, nt * 128 : (nt + 1) * 128],
                        wq_sb[:, c, 2 * C + g * 512 : 2 * C + g * 512 + w],
                        start=(c == 0),
                        stop=(c == CT - 1),
                    )
                    yield
                hview = ps[:, :w].rearrange("p (h d) -> p h d", d=HD)
                dst = v_sb[idx][:, nt, g * 8 : g * 8 + w // HD, 0:HD]
                if copy_engine == "act":
                    nc.scalar.copy(dst, hview)
                else:
                    nc.vector.tensor_copy(dst, hview)

            def emit_proj(idx, nt):
                for _ in gen_proj(idx, nt):
                    pass

            def gen_proj(idx, nt):
                p1 = mmp.tile([128, 512], F32, name="p1", tag="mm")
                p2 = mmp.tile([128, 512], F32, name="p2", tag="mm")
                for c in range(CT):
                    nc.tensor.matmul(
                        p1[:],
                        attnT_sb[idx][:, c, nt * 128 : (nt + 1) * 128],
                        wp_sb[:, c, 0:512],
                        start=(c == 0),
                        stop=(c == CT - 1),
                    )
                    yield
                for c in range(CT):
                    nc.tensor.matmul(
                        p2[:, :256],
                        attnT_sb[idx][:, c, nt * 128 : (nt + 1) * 128],
                        wp_sb[:, c, 512:768],
                        start=(c == 0),
                        stop=(c == CT - 1),
                    )
                    yield
                out_sb = outp.tile([128, C], F32, name="out_sb", tag="outsb")
                nc.vector.tensor_tensor(
                    out_sb[:, 0:512], p1[:], bias_sb[:, 0:512], op=ALU.add
                )
                nc.vector.tensor_tensor(
                    out_sb[:, 512:768], p2[:, :256], bias_sb[:, 512:768], op=ALU.add
                )
                nc.sync.dma_start(out_dram[idx][nt * 128 : (nt + 1) * 128, :], out_sb[:])

            fillers = deque()  # generators yielding once per PE matmul

            def drain_mm(k):
                # advance filler work by k PE matmuls
                while k > 0 and fillers:
                    try:
                        next(fillers[0])
                        k -= 1
                    except StopIteration:
                        fillers.popleft()

            def drain_all():
                while fillers:
                    try:
                        next(fillers[0])
                    except StopIteration:
                        fillers.popleft()

            def emit_norm(idx, t, pvu):
                # batched softmax denominators: gather at partitions 0/32/64/96,
                # one reciprocal, stage each row back to partition 0 (GpSimd
                # partition_broadcast only reads partition 0 on HW)
                keys = list(pvu)
                sums4 = smp.tile([128, 512], F32, name="sums4", tag="sums")
                nc.vector.memset(sums4[:], 1.0)
                for r, k in enumerate(keys):
                    nc.vector.tensor_copy(
                        sums4[32 * r : 32 * r + 1, :], pvu[k][HD : HD + 1, :]
                    )
                recip4 = smp.tile([128, 512], F32, name="recip4", tag="recip")
                nc.vector.reciprocal(recip4[0:97, :], sums4[0:97, :])
                for r, (ab, g) in enumerate(keys):
                    h = 2 * t + ab
                    hc, pb = h // 2, (h % 2) * 64
                    if r == 0:
                        stage = recip4
                    else:
                        stage = smp.tile([1, 512], F32, name=f"st{r}", tag=f"st{r}")
                        nc.vector.tensor_copy(stage[0:1, :], recip4[32 * r : 32 * r + 1, :])
                    rb_sb = rbsbp.tile([64, 512], F32, name="rb_sb", tag="rb")
                    nc.gpsimd.partition_broadcast(rb_sb[:], stage[0:1, :])
                    nc.vector.tensor_tensor(
                        attnT_sb[idx][pb : pb + 64, hc, g * 512 : (g + 1) * 512],
                        pvu[(ab, g)][0:HD, :],
                        rb_sb[:],
                        op=ALU.mult,
                    )

            def emit_attn_pair(idx, t, prev_norm=None, last=False):
                # Two passes over the key tiles, one per 512-col n-half.
                # Within a pass both heads advance together: their
                # contraction-64 score matmuls sit in disjoint PE row groups
                # (partitions 0-63 / 64-127) and adjacent emission makes the
                # hardware run each pair concurrently (measured 2x). A pass
                # holds only 2 P@V accumulators, leaving 2 PSUM slots in the
                # shared pool for the cross-input filler work.
                pvu = {}
                for g in range(2):
                    if g == 1 and prev_norm is not None:
                        # previous pair's normalization sits mid-pair on the
                        # DVE so pair boundaries only carry the pvu copies
                        emit_norm(*prev_norm)
                    pv = {
                        ab: mmp.tile([HD + 1, 512], F32, name="pv", tag="mm")
                        for ab in range(2)
                    }

                    def sc_exp(mt):
                        # one tile holds this n-half's scores for BOTH heads
                        sc = scp.tile([128, 2, 512], F32, name="sc", tag="sc")
                        for ab in range(2):
                            pb = ab * 64
                            nc.tensor.matmul(
                                sc[:, ab, :],
                                qkT_sb[idx][pb : pb + 64, 6 + t, mt * 128 : (mt + 1) * 128],
                                qkT_sb[idx][pb : pb + 64, t, g * 512 : (g + 1) * 512],
                                start=True,
                                stop=True,
                                tile_position=(pb, 0),
                            )
                        pe = pep.tile([128, N], BF16, name="pe", tag="pexp")
                        nc.scalar.activation(
                            pe[:],
                            sc[:].rearrange("p a b -> p (a b)"),
                            AF.Exp,
                            scale=SCALE,
                        )
                        return pe

                    # scores run one key tile ahead of the P@V partials so
                    # the exp stream (the phase bottleneck) never sits behind
                    # P@V or filler matmuls in the in-order PE queue
                    pe_cur = sc_exp(0)
                    for mt in range(NT):
                        pe_next = sc_exp(mt + 1) if mt + 1 < NT else None
                        for ab in range(2):
                            nc.tensor.matmul(
                                pv[ab],
                                v_sb[idx][:, mt, 2 * t + ab, :],
                                pe_cur[:, ab * 512 : (ab + 1) * 512],
                                start=(mt == 0),
                                stop=(mt == NT - 1),
                            )
                        pe_cur = pe_next
                        drain_mm(2)  # smooth background PE work per key tile
                    for ab in range(2):
                        u = pvup.tile([HD + 1, 512], F32, name="pvu", tag="pvu")
                        # last pair: copy on ScalarE (idle by then) so the DVE
                        # queue doesn't delay releasing the PSUM accumulators
                        if last:
                            nc.scalar.copy(u[:], pv[ab][:])
                        else:
                            nc.vector.tensor_copy(u[:], pv[ab][:])
                        pvu[(ab, g)] = u
                    drain_mm(4)
                return (idx, t, pvu)

            # ---- pipelined emission ----
            # prologue: only what attn(x) pair 0 needs -- q/k j-tiles 0 and
            # 6 plus the 512-col v chunks (heads 0-7); the rest of QKV(x)
            # drains as filler inside the attention windows
            emit_qkT(0, 0, "act")
            emit_qkT(0, 6, "act")
            for nt in range(NT):
                emit_v(0, nt, 0, "act")
            # pair t+1's j-tiles head the queue so pair t's drains emit them
            for t in range(1, H // 2):
                fillers.append(gen_qkT(0, t, "dve"))
                fillers.append(gen_qkT(0, 6 + t, "dve"))
            for nt in range(NT):
                fillers.append(gen_v(0, nt, 1, "dve"))
            for jt in range(H):
                fillers.append(gen_qkT(1, jt, "dve"))
            for nt in range(NT):
                fillers.append(gen_v(1, nt, 0, "dve"))
            pend_norm = None
            for t in range(H // 2):
                pend_norm = emit_attn_pair(0, t, prev_norm=pend_norm, last=(t == H // 2 - 1))
            drain_all()
            # attn(y) with v(y) tail chunks + proj(x) drained in
            for nt in range(NT):
                fillers.append(gen_v(1, nt, 1, "dve"))
            for nt in range(NT):
                fillers.append(gen_proj(0, nt))
            for t in range(H // 2):
                pend_norm = emit_attn_pair(1, t, prev_norm=pend_norm, last=(t == H // 2 - 1))
            emit_norm(*pend_norm)
            drain_all()
            for nt in range(NT):
                emit_proj(1, nt)

    nc.compile()
    return nc


_PROGRAM = None


def _get_program():
    global _PROGRAM
    if _PROGRAM is None:
        _PROGRAM = build_program()
    return _PROGRAM


def make_in_maps(x, y, w_qkv, w_proj, b_proj):
    import ml_dtypes

    bf = ml_dtypes.bfloat16
    x = np.asarray(x, np.float32)
    y = np.asarray(y, np.float32)
    xT = np.ascontiguousarray(x.transpose(0, 2, 1)).astype(bf)
    yT = np.ascontiguousarray(y.transpose(0, 2, 1)).astype(bf)
    wqT = np.ascontiguousarray(np.asarray(w_qkv, np.float32).T).astype(bf)
    wpT = np.ascontiguousarray(np.asarray(w_proj, np.float32).T).astype(bf)
    bp = np.ascontiguousarray(np.asarray(b_proj, np.float32).reshape(1, C))
    return [
        {"xT": xT[i], "yT": yT[i], "wqT": wqT, "wpT": wpT, "bp": bp}
        for i in range(N_CORES)
    ]


def kernel(x, y, w_qkv, w_proj, b_proj):
    nc = _get_program()
    in_maps = make_in_maps(x, y, w_qkv, w_proj, b_proj)
    res = bass_utils.run_bass_kernel_spmd(nc, in_maps, core_ids=list(range(N_CORES)))
    xo = np.stack([np.asarray(res.results[i]["out_x"]) for i in range(N_CORES)])
    yo = np.stack([np.asarray(res.results[i]["out_y"]) for i in range(N_CORES)])
    return (xo, yo)
